# revision 36
# baseline (speedup 1.0000x reference)
"""Trainium2 Bass kernel for batched attention with softmax over the query axis.

Reference computation (per batch element b):
    Q = tokens @ Wq; K = tokens @ Wk; V = tokens @ Wv
    S = Q @ K.T                [T(t), T(s)]
    A = softmax(S, axis=t)     (normalizes over the *query* axis per key column)
    out = A @ V                [T, H]

Sharding: pure data parallelism - B=8 batch elements, one per NeuronCore.
The softmax couples queries only within a batch element, so no collectives.

Per-core implementation (fp16 matmul operands, fp32 PSUM accumulation):
  - W_qk = Wq @ Wk.T is built on-chip so scores need one projection
    G = tokens @ W_qk instead of separate Q and K: S = G @ tokens.T.
  - ALL transposes are PLAIN matmuls against a stationary data tile with a
    streaming fp16 identity (out = X.T @ I). Unlike transpose-mode matmuls,
    plain matmuls get the LDWEIGHTS background-buffer pull-ahead, so
    back-to-back 128x128 transposes pace at ~80ns instead of ~215ns.
  - Weights are DVE-cast to fp16 before transposing (fp16 LDW is 2x fp32).
  - DMA order wk -> token stage 0 -> wq -> wv -> stages 1-3 so the two GT
    prerequisites (Wqk and tokT[0:512]) complete at about the same time.
  - tokT tiles for stage sg+1 are emitted after V(sg) so the PE never waits.
  - S tile [s%128, t]: per-chunk PSUM max (overlaps the score matmuls), one
    combined negated max, exp per chunk with accum_out row sums
    (ScalarE), 1/rowsum folded into V rows (DVE).
  - Junk matmuls at t~8us warm the PE HAM clock before the first transpose;
    a dummy exp preloads the ACT exp table set off the critical path.
Engine balance: DVE does casts, PSUM evacuations, reductions, V scaling;
ScalarE does GT evacuations and exps; GpSimd idle.
"""

import numpy as np

import concourse.bass as bass
import concourse.bacc as bacc
import concourse.tile as tile
from concourse import mybir
from concourse.bass_utils import run_bass_kernel_spmd
from concourse.masks import make_identity

B, T, H, E = 8, 2048, 512, 512
P = 128
NT = T // P      # 16 tiles along t / s
NH = H // P      # 4 tiles along h
FD = 512         # matmul moving free dim (one fp32 PSUM bank)
NC_T = T // FD   # 4 free-dim chunks along t
NST = T // FD    # 4 token stage groups (4 t-tiles each)

F32 = mybir.dt.float32
F16 = mybir.dt.float16
AX = mybir.AxisListType
AF = mybir.ActivationFunctionType

N_CORES = 8

NWARM = 22       # junk warm-up matmuls (N=128) bridging to the first wk chunk


def build():
    nc = bacc.Bacc()
    tok_d = nc.declare_dram_parameter("tokens", [T, H], F32, isOutput=False)
    wq_d = nc.declare_dram_parameter("Wq", [H, E], F32, isOutput=False)
    wk_d = nc.declare_dram_parameter("Wk", [H, E], F32, isOutput=False)
    wv_d = nc.declare_dram_parameter("Wv", [H, H], F32, isOutput=False)
    out_d = nc.declare_dram_parameter("out", [T, H], F32, isOutput=True)

    tok_staged = tok_d.rearrange("(sg tt p) h -> sg p tt h", p=P, tt=NT // NST)
    out_tiled = out_d.rearrange("(tt p) h -> tt p h", p=P)

    with tile.TileContext(nc) as tc:
        with (
            tc.tile_pool(name="persist", bufs=1) as pp,
            tc.tile_pool(name="stage", bufs=2) as sp,
            tc.tile_pool(name="ostage", bufs=3) as osp,
            tc.tile_pool(name="stats", bufs=4) as stp,
            tc.tile_pool(name="psum", bufs=8, space=bass.MemorySpace.PSUM) as psp,
        ):
            ident = pp.tile([P, P], F16, tag="ident")
            make_identity(nc, ident[:])
            ident32 = pp.tile([P, P], F32, tag="ident32")
            make_identity(nc, ident32[:])

            # Warm-up: junk matmuls (no DMA dependency) so the PE HAM clock
            # is ramping by the time the first weight chunk lands.
            junk_ps = psp.tile([P, FD], F32, tag="mm", name="junk_ps")
            for i in range(NWARM):
                nc.tensor.matmul(
                    junk_ps[:, 0:P], ident[:], ident[:], start=True, stop=True
                )
            # Preload the exp ACT table set while ScalarE is idle (plain
            # copies exist in every set, so no reload later).
            djunk = stp.tile([P, 1], F32, tag="djunk")
            nc.vector.memset(djunk[:], 0.0)
            dsink = stp.tile([P, 1], F32, tag="dsink")
            nc.scalar.activation(dsink[:], djunk[:], AF.Exp)

            # ---- input DMAs: wk chunks, wq chunks, token stage 0 tiles,
            # wv, token stages 1-3 ----
            wstages = {}
            for name, wd in (("wk", wk_d), ("wq", wq_d)):
                wtiled = wd.rearrange("(hh p) e -> hh p e", p=P)
                for hh in range(NH):
                    ws = sp.tile([P, E], F32, tag="wstage", bufs=8,
                                 name=f"wst_{name}{hh}")
                    nc.sync.dma_start(ws[:], wtiled[hh])
                    wstages[(name, hh)] = ws

            tstages = []
            for sg in range(NST):
                tstage = sp.tile([P, NT // NST, H], F32, tag="tstage", bufs=4,
                                 name=f"tst{sg}")
                tstages.append(tstage)
            for ti in range(NT // NST):
                nc.sync.dma_start(tstages[0][:, ti], tok_staged[0][:, ti])
            nc.sync.dma_start(tstages[1][:], tok_staged[1])
            wv_stage = sp.tile([P, NH, E], F32, tag="wvstage", bufs=1)
            nc.sync.dma_start(wv_stage[:], wv_d.rearrange("(hh p) e -> p hh e", p=P))
            for sg in range(2, NST):
                nc.sync.dma_start(tstages[sg][:], tok_staged[sg])

            # ---- W transposes (plain matmul vs identity) ----
            wT16 = {}
            w16s = {}
            for name in ("wk", "wq"):
                wT = pp.tile([P, NH, E], F16, tag=f"{name}T", name=f"wT_{name}")
                wT16[name] = wT

            def emit_junk(n, key):
                jp = psp.tile([P, FD], F32, tag="mm", name=f"junk_{key}")
                for _ in range(n):
                    nc.tensor.matmul(
                        jp[:, 0:P], ident[:], ident[:], start=True, stop=True
                    )

            def emit_w_cast(name, hh):
                # ScalarE casts the chunk f32->f16 (idle engine, paces with
                # the DMA). All casts are emitted before any ScalarE evac so
                # evac waits never head-of-line block a cast.
                w16 = sp.tile([P, E], F16, tag="w16", bufs=8,
                              name=f"w16_{name}{hh}")
                nc.scalar.copy(w16[:], wstages[(name, hh)][:])
                w16s[(name, hh)] = w16

            def emit_w_tr(name, hh):
                # fp16 plain-matmul transposes at full PE rate; DVE evac.
                ps_tr = psp.tile([P, NH, P], F32, tag="mm",
                                 name=f"tr_{name}{hh}")
                for eb in range(NH):
                    nc.tensor.matmul(
                        ps_tr[:, eb],
                        w16s[(name, hh)][:, eb * P : (eb + 1) * P],
                        ident[:],
                        start=True,
                        stop=True,
                    )
                nc.vector.tensor_copy(
                    wT16[name][:, :, hh * P : (hh + 1) * P], ps_tr[:]
                )

            def emit_wT_chunk(name, hh):
                emit_w_cast(name, hh)
                emit_w_tr(name, hh)

            for hh in range(NH):
                emit_wT_chunk("wk", hh)
                emit_junk(10, f"wk{hh}")
            emit_junk(6, "wkend")

            # ---- token transpose helpers (casts split out so the DVE
            # never serializes cast->evac chains in front of GT) ----
            tokT = pp.tile([P, NH, T], F16, tag="tokT")
            t16s = {}

            def emit_casts(sg):
                # DVE: with V evacuations on ScalarE, the DVE has ~8us of
                # window per stage for 4.4us of cast+evac work.
                for ti in range(NT // NST):
                    tt = sg * (NT // NST) + ti
                    t16 = sp.tile([P, H], F16, tag="t16", bufs=8, name=f"t16_{tt}")
                    nc.vector.tensor_copy(t16[:], tstages[sg][:, ti])
                    t16s[tt] = t16

            def emit_tr_ht(sg, ht):
                # One h-block across all 4 tiles of the stage -> a single
                # [P,512] evacuation; GT's hb-outer pass ht only needs THIS
                # evacuation, so GT starts after the first one, not all four.
                ps_tr = psp.tile([P, FD], F32, tag="mm", name=f"trh{sg}_{ht}")
                for ti in range(NT // NST):
                    tt = sg * (NT // NST) + ti
                    nc.tensor.matmul(
                        ps_tr[:, ti * P : (ti + 1) * P],
                        t16s[tt][:, ht * P : (ht + 1) * P],
                        ident[:],
                        start=True,
                        stop=True,
                    )
                nc.vector.tensor_copy(
                    tokT[:, ht, sg * FD : (sg + 1) * FD], ps_tr[:]
                )

            # ---- wq transposes + Wqk accumulation with one-chunk
            # lookahead: Wqk group hb only needs wq chunk hb's evacuation
            # (plus all of wkT), so running the transposes one chunk ahead
            # hides every PE->ScalarE->PE evacuation hop.
            Wqk = pp.tile([P, NH, H], F16, tag="Wqk")

            def emit_wqk(hb):
                ps = psp.tile([P, FD], F32, tag="mm", name=f"ps_wqk{hb}")
                for eb in range(NH):
                    nc.tensor.matmul(
                        ps[:],
                        wT16["wq"][:, eb, hb * P : (hb + 1) * P],
                        wT16["wk"][:, eb, :],
                        start=(eb == 0),
                        stop=(eb == NH - 1),
                    )
                nc.scalar.copy(Wqk[:, hb, :], ps[:])

            emit_wT_chunk("wq", 0)
            emit_wT_chunk("wq", 1)
            emit_wqk(0)
            emit_wT_chunk("wq", 2)
            emit_wqk(1)
            emit_wT_chunk("wq", 3)
            emit_wqk(2)
            emit_wqk(3)
            emit_casts(0)
            for ht in range(NH):
                emit_tr_ht(0, ht)
            wv16 = pp.tile([P, NH, E], F16, tag="wv16")
            for hh in range(NH):
                nc.vector.tensor_copy(wv16[:, hh], wv_stage[:, hh])

            # ---- per stage: GT chunk -> casts(sg+1) -> V tiles -> TRs(sg+1)
            GT = pp.tile([P, NH, T], F16, tag="GT")
            V = pp.tile([P, NT, H], F16, tag="V")
            for sg in range(NST):
                tch = sg
                gps = [
                    psp.tile([P, FD], F32, tag="mm", name=f"ps_g{gb}_{tch}")
                    for gb in range(NH)
                ]
                for hb in range(NH):
                    for gb in range(NH):
                        nc.tensor.matmul(
                            gps[gb][:],
                            Wqk[:, hb, gb * P : (gb + 1) * P],
                            tokT[:, hb, tch * FD : (tch + 1) * FD],
                            start=(hb == 0),
                            stop=(hb == NH - 1),
                        )
                for gb in range(NH):
                    nc.scalar.copy(GT[:, gb, tch * FD : (tch + 1) * FD], gps[gb][:])
                if sg + 1 < NST:
                    emit_casts(sg + 1)
                for j, st in enumerate(range(sg * NC_T, (sg + 1) * NC_T)):
                    # next stage's transpose h-block rides ahead of each V
                    # group so its DVE evacuation finishes inside the
                    # V-matmul window (no trailing evac before GT).
                    if sg + 1 < NST:
                        emit_tr_ht(sg + 1, j)
                    ps = psp.tile([P, FD], F32, tag="mm", name=f"ps_v{st}")
                    for ht in range(NH):
                        nc.tensor.matmul(
                            ps[:],
                            tokT[:, ht, st * P : (st + 1) * P],
                            wv16[:, ht, :],
                            start=(ht == 0),
                            stop=(ht == NH - 1),
                        )
                    nc.scalar.copy(V[:, st, :], ps[:])

            # ---- scores S[s,t] + softmax over t (free axis) ----
            Etile = pp.tile([P, NT, T], F16, tag="E")
            for st in range(NT):
                pss = [
                    psp.tile([P, FD], F32, tag="mm", name=f"ps_s{st}_{tch}")
                    for tch in range(NC_T)
                ]
                mx4 = stp.tile([P, NC_T], F32, tag="mx4")
                for tch in range(NC_T):
                    for hb in range(NH):
                        nc.tensor.matmul(
                            pss[tch][:],
                            tokT[:, hb, st * P : (st + 1) * P],
                            GT[:, hb, tch * FD : (tch + 1) * FD],
                            start=(hb == 0),
                            stop=(hb == NH - 1),
                        )
                    nc.vector.reduce_max(
                        mx4[:, tch : tch + 1], pss[tch][:], axis=AX.X
                    )
                nmx = stp.tile([P, 1], F32, tag="nmx")
                nc.vector.reduce_max(nmx[:], mx4[:], axis=AX.X, negate=True)
                racc = stp.tile([P, NC_T], F32, tag="racc")
                for tch in range(NC_T):
                    nc.scalar.activation(
                        Etile[:, st, tch * FD : (tch + 1) * FD],
                        pss[tch][:],
                        AF.Exp,
                        bias=nmx[:],
                        accum_out=racc[:, tch : tch + 1],
                    )
                rsum = stp.tile([P, 1], F32, tag="rsum")
                nc.vector.reduce_sum(rsum[:], racc[:], axis=AX.X)
                rinv = stp.tile([P, 1], F32, tag="rinv")
                nc.vector.reciprocal(rinv[:], rsum[:])
                # Fold 1/rowsum into V rows (rowsum is per-s, V is s-major).
                nc.vector.tensor_scalar_mul(V[:, st, :], V[:, st, :], rinv[:])

            # ---- context: ctx[t,h] = sum_s E[s,t] * V'[s,h] ----
            for tt in range(NT - 1):
                ps = psp.tile([P, FD], F32, tag="mm", name=f"ps_c{tt}")
                for st in range(NT):
                    nc.tensor.matmul(
                        ps[:],
                        Etile[:, st, tt * P : (tt + 1) * P],
                        V[:, st, :],
                        start=(st == 0),
                        stop=(st == NT - 1),
                    )
                ot = osp.tile([P, H], F32, tag="ostage", name=f"ost{tt}")
                if tt == NT - 2:
                    # tt14 drains on the (by now nearly empty) sync ring in
                    # parallel with tt15 on the scalar ring.
                    for cc in range(2):
                        sl = slice(cc * (H // 2), (cc + 1) * (H // 2))
                        nc.vector.tensor_copy(ot[:, sl], ps[:, sl])
                        nc.sync.dma_start(out_tiled[tt][:, sl], ot[:, sl])
                else:
                    nc.vector.tensor_copy(ot[:], ps[:])
                    nc.sync.dma_start(out_tiled[tt], ot[:])
            # Last tile as two half-width accumulation chains so the final
            # evac+DMA overlaps the second half's matmuls.
            tt = NT - 1
            ot = osp.tile([P, H], F32, tag="ostage", name=f"ost{tt}")
            for cc, (lo, hi) in enumerate(((0, 256), (256, 384), (384, 512))):
                sl = slice(lo, hi)
                ps = psp.tile([P, hi - lo], F32, tag="mm", name=f"ps_c{tt}_{cc}")
                for st in range(NT):
                    nc.tensor.matmul(
                        ps[:],
                        Etile[:, st, tt * P : (tt + 1) * P],
                        V[:, st, lo:hi],
                        start=(st == 0),
                        stop=(st == NT - 1),
                    )
                nc.vector.tensor_copy(ot[:, sl], ps[:])
                nc.scalar.dma_start(out_tiled[tt][:, sl], ot[:, sl])

    nc.compile()
    return nc


_NC = None


def _get_nc():
    global _NC
    if _NC is None:
        _NC = build()
    return _NC


def _run(inputs, trace=False, **kwargs):
    tokens = np.ascontiguousarray(inputs["tokens"], dtype=np.float32)
    Wq = np.ascontiguousarray(inputs["Wq"], dtype=np.float32)
    Wk = np.ascontiguousarray(inputs["Wk"], dtype=np.float32)
    Wv = np.ascontiguousarray(inputs["Wv"], dtype=np.float32)
    assert tokens.shape == (B, T, H)
    nc = _get_nc()
    in_maps = [
        {"tokens": tokens[i], "Wq": Wq, "Wk": Wk, "Wv": Wv} for i in range(N_CORES)
    ]
    res = run_bass_kernel_spmd(
        nc, in_maps, core_ids=list(range(N_CORES)), trace=trace, **kwargs
    )
    out = np.stack([res.results[i]["out"] for i in range(N_CORES)], axis=0)
    return out.astype(np.float32), res


def kernel(**inputs) -> np.ndarray:
    out, _ = _run(inputs)
    return out
